# revision 14
# baseline (speedup 1.0000x reference)
"""Trainium2 Bass kernel for nn_DecoderRNN (pointer-generator GRU decoder).

Sharding (8 NeuronCores, zero cross-core communication):
  - Phase A (63-step recurrence): data-parallel over batch, 4 sequences per
    core. GRU / attention / concat projection as small matmuls with weights
    stationary and (feature-tile x batch) packed on the free axis.
  - Phase B: softmax(c_t @ out_W.T + out_b) rows for the core's own
    (step, batch) rows, streaming out_W from HBM in 500-wide chunks.
  - Host: embedding gather, weight layouts, pointer-copy scatter-add.
"""
import os
import numpy as np

import concourse.bass as bass
import concourse.bass_isa as bass_isa
import concourse.bacc as bacc
import concourse.mybir as mybir
import concourse.tile as tile
from concourse import bass_utils

mdt = mybir.dt
AF = mybir.ActivationFunctionType
ALU = mybir.AluOpType
AX = mybir.AxisListType

N_CORES = 8
B, S, H, E, V, OOV = 32, 128, 1024, 512, 32000, 20
VO = V + OOV
IDX_UNK = 2
BC = B // N_CORES          # 4 batch rows per core
NCHUNK = 500
NCH = V // NCHUNK          # 64 vocab chunks

_f32 = mdt.float32
_bf16 = mdt.bfloat16
_np_bf16 = mdt.np(_bf16)


def _groups(T):
    gs, t0 = [], 0
    while t0 < T:
        nt = min(32, T - t0)
        gs.append((t0, nt))
        t0 += nt
    return gs


def build_program(T, ptr_b_val):
    nc = bacc.Bacc("TRN2", target_bir_lowering=False, debug=False,
                   num_devices=N_CORES)
    NT = BC * T
    ngrp = len(_groups(T))

    def din(name, shape, dtype):
        return nc.dram_tensor(name, shape, dtype, kind="ExternalInput")

    xembT = din("xembT", [E, NT], _bf16)
    encTd = din("encT", [128, BC * 8 * 128], _f32)
    encNd = din("encN", [128, BC * 8 * 128], _bf16)
    WxT = din("WxT", [E, 3 * H], _bf16)
    WcrzT = din("WcrzT", [H, 2 * H], _bf16)
    WhrzT = din("WhrzT", [H, 2 * H], _bf16)
    WcnT = din("WcnT", [H, H], _bf16)
    WhnT = din("WhnT", [H, H], _bf16)
    WccT = din("WccT", [2 * H, H], _bf16)
    outWT = din("outWT", [H, V], _bf16)
    outbd = din("outb", [1, V], _bf16)
    ptrWT = din("ptrWT", [128, 20], _bf16)
    bgix = din("bgix", [128, 24], _f32)
    bnh32 = din("bnh32", [128, 32], _f32)
    bcc32 = din("bcc32", [128, 32], _f32)
    maskT = din("maskT", [128, BC], _f32)
    h0pack = din("h0pack", [128, 32], _f32)

    pout = nc.dram_tensor("pout", [T * BC, VO], _f32, kind="ExternalOutput")
    pgout = nc.dram_tensor("pgout", [NT, 1], _f32, kind="ExternalOutput")
    aout = nc.dram_tensor("aout", [128, NT], _f32, kind="ExternalOutput")

    with tile.TileContext(nc) as tc:
        with tc.tile_pool(name="persist", bufs=1) as pp:
            # ---- persistent SBUF ----
            c_store = pp.tile([128, 32 * (T + 1)], _bf16, tag="c_store")
            h_store = pp.tile([128, 32 * (T + 1)], _bf16, tag="h_store")
            ctx_store = pp.tile([128, 32 * T], _bf16, tag="ctx_store")
            attn_store = pp.tile([128, NT], _f32, tag="attn_store")
            h_state = pp.tile([128, 32], _f32, tag="h_state")
            xemb_sb = pp.tile([128, 4 * NT], _bf16, tag="xembT")
            ptrW_sb = pp.tile([128, 20], _bf16, tag="ptrW")
            bgix_sb = pp.tile([128, 24], _f32, tag="bgix")
            bnh_sb = pp.tile([128, 32], _f32, tag="bnh")
            bcc_sb = pp.tile([128, 32], _f32, tag="bcc")
            mask_sb = pp.tile([128, BC], _f32, tag="mask")
            pg_sb = pp.tile([128, ngrp], _f32, tag="pg")

            nc.sync.dma_start(
                xemb_sb[:].rearrange("p (kt n) -> p kt n", kt=4),
                xembT.ap().rearrange("(kt p) n -> p kt n", p=128))
            nc.sync.dma_start(ptrW_sb[:], ptrWT[:])
            nc.sync.dma_start(bgix_sb[:], bgix[:])
            nc.sync.dma_start(bnh_sb[:], bnh32[:])
            nc.sync.dma_start(bcc_sb[:], bcc32[:])
            nc.sync.dma_start(mask_sb[:], maskT[:])
            nc.sync.dma_start(h_state[:], h0pack[:])

            c_r = c_store[:].rearrange("p (t x) -> p t x", x=32)
            h_r = h_store[:].rearrange("p (t x) -> p t x", x=32)
            x_r = ctx_store[:].rearrange("p (t x) -> p t x", x=32)

            with (
                tc.tile_pool(name="weights", bufs=1) as pw,
                tc.tile_pool(name="psum", bufs=1, space="PSUM") as ps,
            ):
                encT = pw.tile([128, BC * 8 * 128], _f32, tag="encT")
                encN = pw.tile([128, BC * 8 * 128], _bf16, tag="encN")
                wcrz = pw.tile([128, 8 * 2048], _bf16, tag="wcrz")
                whrz = pw.tile([128, 8 * 2048], _bf16, tag="whrz")
                wcn = pw.tile([128, 8 * 1024], _bf16, tag="wcn")
                whn = pw.tile([128, 8 * 1024], _bf16, tag="whn")
                wcc = pw.tile([128, 16 * 1024], _bf16, tag="wcc")
                gix = pw.tile([128, 96 * T], _bf16, tag="gix")
                nc.sync.dma_start(encT[:], encTd[:])
                nc.sync.dma_start(encN[:], encNd[:])
                for name, tl, src, nkt in (("crz", wcrz, WcrzT, 8),
                                           ("hrz", whrz, WhrzT, 8),
                                           ("cn", wcn, WcnT, 8),
                                           ("hn", whn, WhnT, 8),
                                           ("cc", wcc, WccT, 16)):
                    nc.sync.dma_start(
                        tl[:].rearrange("p (kt m) -> p kt m", kt=nkt),
                        src.ap().rearrange("(kt p) m -> p kt m", p=128))
                g_r = gix[:].rearrange("p (t x) -> p t x", x=96)

                # ---- phase 0: gix = x_emb @ Wx.T + fused biases ----
                with nc.named_scope("phase0"):
                    nc.vector.memset(c_store[:, 0:32], 0.0)
                    nc.vector.tensor_copy(h_store[:, 0:32], h_state[:])
                    with tc.tile_pool(name="p0", bufs=1) as p0:
                        for half in range(2):
                            wx_sb = p0.tile([128, 4 * 1536], _bf16, tag="wx")
                            nc.sync.dma_start(
                                wx_sb[:].rearrange("p (kt m) -> p kt m", kt=4),
                                WxT.ap()[:, half * 1536:(half + 1) * 1536]
                                .rearrange("(kt p) m -> p kt m", p=128))
                            for mi in range(12):
                                m = half * 12 + mi
                                pg0 = ps.tile([128, NT], _f32, tag="ps_g")
                                for kt in range(4):
                                    nc.tensor.matmul(
                                        pg0[:, :NT],
                                        wx_sb[:, kt * 1536 + mi * 128:
                                              kt * 1536 + (mi + 1) * 128],
                                        xemb_sb[:, kt * NT:(kt + 1) * NT],
                                        start=(kt == 0), stop=(kt == 3))
                                nc.vector.tensor_scalar_add(
                                    g_r[:, :, 4 * m:4 * m + 4],
                                    pg0[:, :NT].rearrange(
                                        "p (t x) -> p t x", x=4),
                                    bgix_sb[:, m:m + 1])

                # ---- phase A ----
                with nc.named_scope("phaseA"):
                    cp0 = pp.tile([128, 32], _bf16, tag="cp0")
                    cp1 = pp.tile([128, 32], _bf16, tag="cp1")
                    hp0 = pp.tile([128, 32], _bf16, tag="hp0")
                    hp1 = pp.tile([128, 32], _bf16, tag="hp1")
                    cp_t = [cp0, cp1]
                    hp_t = [hp0, hp1]
                    nc.vector.memset(cp_t[0][:], 0.0)
                    nc.vector.tensor_copy(hp_t[0][:], h_state[:])
                    parity = [0]

                    def step_body(t):
                        pr = parity[0]
                        parity[0] = 1 - pr
                        c_prev, h_prev = cp_t[pr], hp_t[pr]
                        c_cur, h_cur = cp_t[1 - pr], hp_t[1 - pr]
                        p_rz = ps.tile([128, 64], _f32, tag="ps_rz")
                        for m in range(16):
                            for i_, kt in enumerate(
                                    list(range(8, 16)) + list(range(8))):
                                w = wcrz if kt < 8 else whrz
                                st = c_prev if kt < 8 else h_prev
                                nc.tensor.matmul(
                                    p_rz[:, 4 * m:4 * m + 4],
                                    w[:, (kt % 8) * 2048 + m * 128:
                                      (kt % 8) * 2048 + (m + 1) * 128],
                                    st[:, 4 * (kt % 8):4 * (kt % 8) + 4],
                                    start=(i_ == 0), stop=(i_ == 15))
                        rz1 = pp.tile([128, 64], _f32, tag="rz1")
                        nc.vector.tensor_add(rz1[:], p_rz[:],
                                             gix[:, bass.ds(96 * t, 64)])
                        nc.scalar.activation(rz1[:], rz1[:], AF.Sigmoid)

                        p_in = ps.tile([128, 32], _f32, tag="ps_in")
                        p_hn = ps.tile([128, 32], _f32, tag="ps_hn")
                        for m in range(8):
                            for kt in range(8):
                                nc.tensor.matmul(
                                    p_hn[:, 4 * m:4 * m + 4],
                                    whn[:, kt * 1024 + m * 128:
                                        kt * 1024 + (m + 1) * 128],
                                    h_prev[:, 4 * kt:4 * kt + 4],
                                    start=(kt == 0), stop=(kt == 7))
                        for m in range(8):
                            for kt in range(8):
                                nc.tensor.matmul(
                                    p_in[:, 4 * m:4 * m + 4],
                                    wcn[:, kt * 1024 + m * 128:
                                        kt * 1024 + (m + 1) * 128],
                                    c_prev[:, 4 * kt:4 * kt + 4],
                                    start=(kt == 0), stop=(kt == 7))
                        t1 = pp.tile([128, 32], _f32, tag="t1")
                        nc.vector.tensor_add(t1[:], p_hn[:], bnh_sb[:])
                        nc.vector.tensor_mul(t1[:], t1[:], rz1[:, 0:32])
                        t3 = pp.tile([128, 32], _f32, tag="t3")
                        nc.vector.tensor_add(t3[:], p_in[:],
                                             gix[:, bass.ds(96 * t + 64, 32)])
                        nc.vector.tensor_add(t3[:], t3[:], t1[:])
                        nc.scalar.activation(t3[:], t3[:], AF.Tanh)
                        hmn = pp.tile([128, 32], _f32, tag="hmn")
                        nc.vector.tensor_sub(hmn[:], h_state[:], t3[:])
                        nc.vector.tensor_mul(hmn[:], hmn[:], rz1[:, 32:64])
                        nc.vector.tensor_add(h_state[:], t3[:], hmn[:])
                        nc.vector.tensor_copy(h_cur[:], h_state[:])
                        nc.vector.tensor_copy(
                            h_store[:, bass.ds(32 * t + 32, 32)], h_cur[:])

                        p_sc = ps.tile([128, BC], _f32, tag="ps_sc")
                        for j in range(BC):
                            for k in range(8):
                                nc.tensor.matmul(
                                    p_sc[:, j:j + 1],
                                    encT[:, (j * 8 + k) * 128:
                                         (j * 8 + k + 1) * 128],
                                    h_state[:, 4 * k + j:4 * k + j + 1],
                                    start=(k == 0), stop=(k == 7))
                        sc1 = pp.tile([128, BC], _f32, tag="sc1")
                        nc.vector.tensor_copy(sc1[:], p_sc[:])
                        mx = pp.tile([128, BC], _f32, tag="mx")
                        nc.gpsimd.partition_all_reduce(
                            mx[:], sc1[:], channels=128,
                            reduce_op=bass_isa.ReduceOp.max)
                        nc.vector.tensor_tensor(sc1[:], sc1[:], mx[:],
                                                op=ALU.subtract)
                        nc.scalar.activation(sc1[:], sc1[:], AF.Exp)
                        nc.vector.tensor_mul(sc1[:], sc1[:], mask_sb[:])
                        sm = pp.tile([128, BC], _f32, tag="sm")
                        nc.gpsimd.partition_all_reduce(
                            sm[:], sc1[:], channels=128,
                            reduce_op=bass_isa.ReduceOp.add)
                        nc.vector.reciprocal(sm[:], sm[:])
                        nc.vector.tensor_tensor(
                            attn_store[:, bass.ds(BC * t, BC)], sc1[:],
                            sm[:], op=ALU.mult)
                        ab = pp.tile([128, BC], _bf16, tag="ab")
                        nc.vector.tensor_copy(
                            ab[:], attn_store[:, bass.ds(BC * t, BC)])

                        p_cx = ps.tile([128, 32], _f32, tag="ps_cx")
                        for m in range(8):
                            for j in range(BC):
                                nc.tensor.matmul(
                                    p_cx[:, 4 * m + j:4 * m + j + 1],
                                    encN[:, (j * 8 + m) * 128:
                                         (j * 8 + m + 1) * 128],
                                    ab[:, j:j + 1], start=True, stop=True)
                        cxb = pp.tile([128, 32], _bf16, tag="cxb")
                        nc.vector.tensor_copy(cxb[:], p_cx[:])
                        nc.vector.tensor_copy(
                            ctx_store[:, bass.ds(32 * t, 32)], cxb[:])

                        p_cc = ps.tile([128, 32], _f32, tag="ps_cc")
                        for m in range(8):
                            for kt in range(16):
                                if kt < 8:
                                    rhs = h_cur[:, 4 * kt:4 * kt + 4]
                                else:
                                    rhs = cxb[:, 4 * (kt - 8):4 * (kt - 8) + 4]
                                nc.tensor.matmul(
                                    p_cc[:, 4 * m:4 * m + 4],
                                    wcc[:, kt * 1024 + m * 128:
                                        kt * 1024 + (m + 1) * 128],
                                    rhs, start=(kt == 0), stop=(kt == 15))
                        cc1 = pp.tile([128, 32], _f32, tag="cc1")
                        nc.vector.tensor_add(cc1[:], p_cc[:], bcc_sb[:])
                        nc.scalar.activation(c_cur[:], cc1[:], AF.Tanh)
                        nc.vector.tensor_copy(
                            c_store[:, bass.ds(32 * t + 32, 32)], c_cur[:])

                    nloop = T if T % 2 == 0 else T - 1
                    if nloop > 0:
                        with tc.For_i(0, nloop, 2,
                                      hint_engines=(
                                          mybir.EngineType.PE,
                                          mybir.EngineType.DVE,
                                          mybir.EngineType.Activation)) as t:
                            step_body(t)
                            step_body(t + 1)
                    if T % 2 == 1:
                        step_body(T - 1)

                # ---- p_gen (uses ctx/h/xemb stores) ----
                groups = _groups(T)
                with nc.named_scope("pgen"):
                    ptrb_sb = pp.tile([128, 1], _f32, tag="ptrb")
                    nc.vector.memset(ptrb_sb[:], float(ptr_b_val))
                    pg_lhs = pp.tile([128, 20 * 128], _bf16, tag="pg_lhs")
                    for gidx, (t0, nt) in enumerate(groups):
                        nr_ = 4 * nt
                        p_pg = ps.tile([128, 1], _f32, tag="ps_pg")
                        for k in range(20):
                            if k < 8:
                                lhs = x_r[:, t0:t0 + nt, 4 * k:4 * k + 4]
                            elif k < 16:
                                lhs = h_r[:, t0 + 1:t0 + 1 + nt,
                                          4 * (k - 8):4 * (k - 8) + 4]
                            else:
                                lhs = xemb_sb[:, (k - 16) * NT + 4 * t0:
                                              (k - 16) * NT + 4 * (t0 + nt)]
                            nc.vector.tensor_copy(
                                pg_lhs[:, k * 128:k * 128 + nr_], lhs)
                        for k in range(20):
                            nc.tensor.matmul(p_pg[:nr_, :1],
                                             pg_lhs[:, k * 128:k * 128 + nr_],
                                             ptrW_sb[:, k:k + 1],
                                             start=(k == 0), stop=(k == 19))
                        nc.scalar.activation(pg_sb[:4 * nt, gidx:gidx + 1],
                                             p_pg[:4 * nt, :1], AF.Sigmoid,
                                             bias=ptrb_sb[:4 * nt, :])
                        nc.sync.dma_start(pgout.ap()[4 * t0:4 * (t0 + nt), :],
                                          pg_sb[:4 * nt, gidx:gidx + 1])
                nc.sync.dma_start(aout.ap(), attn_store[:])

            # ---- phase B ----
            groups = _groups(T)
            with nc.named_scope("phaseB"):
                outw_r = outWT.ap().rearrange("(kt p) v -> p kt v", p=128)
                pout_a = pout.ap()
                with (tc.tile_pool(name="pb", bufs=1) as pb,
                      tc.tile_pool(name="pb2", bufs=2) as pb2,
                      tc.tile_pool(name="psumB", bufs=4, space="PSUM") as psb):
                    ones_row = pb.tile([1, 128], _bf16, tag="ones_row")
                    nc.vector.memset(ones_row[:], 1.0)
                    with tc.tile_pool(name="pbl", bufs=3) as pbl:
                        for gidx, (t0, nt) in enumerate(groups):
                            nr = 4 * nt
                            m_store = pb2.tile([128, V], _bf16, tag="m_store")
                            csum = pb2.tile([128, NCH], _f32, tag="csum")
                            rs = pb2.tile([128, 1], _f32, tag="rs")
                            cb_g = pb2.tile([128, 8 * 128], _bf16, tag="cb_g")
                            for kt in range(8):
                                nc.vector.tensor_copy(
                                    cb_g[:, kt * 128:kt * 128 + nr],
                                    c_r[:, t0 + 1:t0 + 1 + nt,
                                        4 * kt:4 * kt + 4])
                            for ch in range(NCH):
                                rhs_t = pbl.tile([128, 8 * NCHUNK], _bf16,
                                                 tag="rhs")
                                nc.sync.dma_start(
                                    rhs_t[:].rearrange(
                                        "p (kt v) -> p kt v", v=NCHUNK),
                                    outw_r[:, :,
                                           ch * NCHUNK:(ch + 1) * NCHUNK])
                                outb_t = pbl.tile([1, NCHUNK], _bf16,
                                                  tag="outb_t")
                                nc.sync.dma_start(
                                    outb_t[:],
                                    outbd.ap()[:, ch * NCHUNK:
                                               (ch + 1) * NCHUNK])
                                p_b = psb.tile([128, NCHUNK], _f32, tag="ps_b")
                                for kt in range(8):
                                    nc.tensor.matmul(
                                        p_b[:nr, :],
                                        cb_g[:, kt * 128:kt * 128 + nr],
                                        rhs_t[:, kt * NCHUNK:
                                              (kt + 1) * NCHUNK],
                                        start=(kt == 0), stop=False)
                                nc.tensor.matmul(
                                    p_b[:nr, :], ones_row[:, :nr],
                                    outb_t[:, :], start=False, stop=True)
                                nc.scalar.activation(
                                    m_store[:nr, ch * NCHUNK:
                                            (ch + 1) * NCHUNK],
                                    p_b[:nr, :], AF.Exp,
                                    accum_out=csum[:nr, ch:ch + 1])
                            nc.vector.reduce_sum(rs[:nr, :], csum[:nr, :],
                                                 axis=AX.X)
                            nc.vector.reciprocal(rs[:nr, :], rs[:nr, :])
                            nc.vector.tensor_mul(rs[:nr, :], rs[:nr, :],
                                                 pg_sb[:nr, gidx:gidx + 1])
                            for ch in range(NCH):
                                o_t = pbl.tile([128, NCHUNK], _f32, tag="o_t")
                                nc.scalar.activation(
                                    o_t[:nr, :],
                                    m_store[:nr, ch * NCHUNK:
                                            (ch + 1) * NCHUNK],
                                    AF.Copy, scale=rs[:nr, :])
                                nc.sync.dma_start(
                                    pout_a[4 * t0:4 * t0 + nr,
                                           ch * NCHUNK:(ch + 1) * NCHUNK],
                                    o_t[:nr, :])
    nc.compile()
    return nc


def host_prep(T, targets, h0, encoder_output, input_lens,
              emb_W, rnn_W_ih, rnn_W_hh, rnn_b_ih, rnn_b_hh,
              concat_W, concat_b, out_W, out_b, ptr_W):
    f32 = np.float32
    tok = np.asarray(targets)[:, :T]
    tok = np.where(tok >= V, IDX_UNK, tok)
    emb_W = np.asarray(emb_W, f32)
    x_emb = emb_W[tok]                      # (B, T, E)
    enc = np.asarray(encoder_output, f32)
    W_ih = np.asarray(rnn_W_ih, f32)
    W_hh = np.asarray(rnn_W_hh, f32)
    b_ih = np.asarray(rnn_b_ih, f32)
    b_hh = np.asarray(rnn_b_hh, f32)
    cW = np.asarray(concat_W, f32)
    cb = np.asarray(concat_b, f32)
    oW = np.asarray(out_W, f32)
    ob = np.asarray(out_b, f32)
    pW = np.asarray(ptr_W, f32)
    lens = np.asarray(input_lens)
    h0 = np.asarray(h0, f32)

    bf = lambda a: np.ascontiguousarray(a).astype(_np_bf16)

    shared = {
        "WxT": bf(W_ih[:, :E].T),
        "WcrzT": bf(W_ih[:2 * H, E:].T),
        "WhrzT": bf(W_hh[:2 * H, :].T),
        "WcnT": bf(W_ih[2 * H:, E:].T),
        "WhnT": bf(W_hh[2 * H:, :].T),
        "WccT": bf(cW.T),
        "outWT": bf(oW.T),
        "outb": bf(ob[None, :]),
        "ptrWT": bf(pW[0].reshape(20, 128).T),
        "bgix": np.ascontiguousarray(np.concatenate(
            [(b_ih[:2 * H] + b_hh[:2 * H]).reshape(16, 128).T,
             b_ih[2 * H:].reshape(8, 128).T], axis=1)).astype(f32),
        "bnh32": np.ascontiguousarray(
            np.repeat(b_hh[2 * H:].reshape(8, 128).T, 4, axis=1)).astype(f32),
        "bcc32": np.ascontiguousarray(
            np.repeat(cb.reshape(8, 128).T, 4, axis=1)).astype(f32),
    }
    in_maps = []
    for c in range(N_CORES):
        bs = slice(BC * c, BC * (c + 1))
        xe = x_emb[bs]                       # (4, T, E)
        xembT_ = xe.transpose(2, 1, 0).reshape(E, T * BC)
        encc = enc[bs]                       # (4, S, H)
        encT_ = encc.transpose(0, 2, 1).reshape(BC, 8, 128, S) \
                    .transpose(2, 0, 1, 3).reshape(128, BC * 8 * 128)
        encN_ = encc.reshape(BC, S, 8, 128).transpose(1, 0, 2, 3) \
                    .reshape(S, BC * 8 * 128)
        maskT_ = (np.arange(S)[:, None] < lens[bs][None, :]).astype(f32)
        h0p = h0[0, bs].reshape(BC, 8, 128).transpose(2, 1, 0).reshape(128, 32)
        m = dict(shared)
        m.update({
            "xembT": bf(xembT_),
            "encT": np.ascontiguousarray(encT_).astype(f32),
            "encN": bf(encN_),
            "maskT": np.ascontiguousarray(maskT_),
            "h0pack": np.ascontiguousarray(h0p),
        })
        in_maps.append(m)
    return in_maps


_prog_cache = {}


def _run(nc, in_maps):
    if os.environ.get("KBENCH_SIM"):
        from concourse.bass_interp import MultiCoreSim
        sim = MultiCoreSim(nc, num_cores=N_CORES, trace=False)
        cores = list(sim.cores.values())
        for c, core in enumerate(cores):
            for k, v in in_maps[c].items():
                core.tensor(k)[:] = v
        sim.simulate(check_with_hw=False)
        return [{n: np.array(core.tensor(n))
                 for n in ("pout", "pgout", "aout")} for core in cores]
    res = bass_utils.run_bass_kernel_spmd(
        nc, in_maps, core_ids=list(range(N_CORES)),
        trace=bool(os.environ.get("KBENCH_TRACE")))
    if os.environ.get("KBENCH_TRACE"):
        print("HW exec time:", res.exec_time_ns, "ns")
        print("scope times:", res.per_core_scope_times)
    return res.results


def kernel(targets, h0, encoder_output, inputs, input_lens, oov_size,
           emb_W, rnn_W_ih, rnn_W_hh, rnn_b_ih, rnn_b_hh,
           concat_W, concat_b, out_W, out_b, ptr_W, ptr_b):
    T = np.asarray(targets).shape[1] - 1
    ptr_b_val = float(np.asarray(ptr_b).reshape(-1)[0])
    if T not in _prog_cache:
        _prog_cache[T] = build_program(T, ptr_b_val)
    nc = _prog_cache[T]

    in_maps = host_prep(T, targets, h0, encoder_output, input_lens,
                        emb_W, rnn_W_ih, rnn_W_hh, rnn_b_ih, rnn_b_hh,
                        concat_W, concat_b, out_W, out_b, ptr_W)
    results = _run(nc, in_maps)

    outs = np.zeros((T, B, VO), np.float32)
    pgens = np.empty((T, B, 1), np.float32)
    attn = np.empty((T, B, S), np.float32)
    for c in range(N_CORES):
        r = results[c]
        bs = slice(BC * c, BC * (c + 1))
        outs[:, bs, :V] = r["pout"].reshape(T, BC, VO)[:, :, :V]
        pgens[:, bs, 0] = r["pgout"].reshape(T, BC)
        attn[:, bs, :] = r["aout"].reshape(S, T, BC).transpose(1, 2, 0)

    inp = np.asarray(inputs)
    p_copy = (1.0 - pgens) * attn            # (T, B, S)
    base = (np.arange(T)[:, None, None] * B
            + np.arange(B)[None, :, None]) * VO
    flat_idx = (base + inp[None, :, :]).ravel()
    np.add.at(outs.reshape(-1), flat_idx, p_copy.ravel())
    return outs, pgens
